# revision 36
# baseline (speedup 1.0000x reference)
"""Multi-head attention (B=4, S=2048, H=16, d_model=1024, d_k=d_v=64) on 8
Trainium2 NeuronCores.

Sharding: 8 cores = 4 batches x 2 query-halves. Each core computes all 16
heads for its (batch, query-half); K/V projections are recomputed per
query-half so no inter-core communication is needed; outputs are disjoint
and concatenated on the host.

Host prep: Q/K/V are transposed to [d_model, seq] (V additionally blocked
by s-chunk) and cast to bf16 on the host, as are all weights, so the kernel
needs no on-chip transposes of the activations.

Per-core pipeline:
  - projections: kt[p] = W_K[pair p].T @ K -> SBUF [128, S] bf16 per pair;
    qt[p] likewise [128, QH]; v_all = per s-chunk blocks of 16 head-slots
    [1|v] (65 wide, bf16)
  - attention per pair, software-pipelined: only pair 0's K/Q projections
    run up front; all V projection groups (quarter-width, N=256) and the
    remaining K/Q pairs are emitted as PE filler work inside the attention
    loop (which is otherwise paced by the ACT engine's exp).
    scoresT = kt-chunk.T @ qt (PSUM f32), e = exp(s/8) on ACT -> bf16, then
    the flipped value matmul o[q, 65] = e-chunk.T @ [1|v] (moving operand
    only 65 columns) accumulated over s-chunks; column 0 is the softmax
    denominator.
  - normalize: one broadcast multiply by 1/denom into the pair's dead kt
    tile, then PE re-transpose (x identity) into the pair's qt tile as
    pair-stacked headsT [128, QH] (scheduled as filler in pairs 6-7).
  - output projection: out = concat(heads) @ W_O accumulated over 8
    pair-chunks.

PSUM note: start_tensor_calc marks the whole 2KB bank pending-zero, so
interleaved per-slot accumulation groups sharing a bank must issue exactly
one start (first slot); the other slots' first writes land on pending-zero
bytes, which the hardware treats as overwrite.
"""

import contextlib
import os
import sys

for _p in ("/opt/trn_rl_repo", "/root/.axon_site/_ro/trn_rl_repo"):
    if os.path.isdir(_p) and _p not in sys.path:
        sys.path.insert(0, _p)

import numpy as np
import ml_dtypes

import concourse.bass as bass  # noqa: F401
import concourse.tile as tile
from concourse import bacc, mybir
from concourse.bass_utils import run_bass_kernel_spmd
from concourse.masks import make_identity

F32 = mybir.dt.float32
BF16 = mybir.dt.bfloat16

B, S, DM = 4, 2048, 1024
H, D = 16, 64
QH = S // 2  # query half per core
N_CORES = 8
NP = H // 2  # head pairs
N_SC = S // 128  # kv 128-chunks
N_MO = DM // 128  # model-dim 128-chunks
N_QC = QH // 128  # q 128-chunks


def build(n_cores=N_CORES, phases=(1, 2, 3), dbg=False):
    nc = bacc.Bacc("TRN2", target_bir_lowering=False, debug=False, num_devices=n_cores)

    # host-transposed activations, bf16
    qt_d = nc.dram_tensor("QT", [N_MO, 128, QH], BF16, kind="ExternalInput").ap()
    kt_d = nc.dram_tensor("KT", [N_MO, 128, S], BF16, kind="ExternalInput").ap()
    # V blocked by s-chunk: [sc, 128(dm within mo), mo, 128(s within chunk)]
    vt_d = nc.dram_tensor(
        "VTs", [N_SC, 128, N_MO, 128], BF16, kind="ExternalInput"
    ).ap()
    # host-prepped weights, bf16; K/Q pair-major: [pair, mi=128, mo=8, 128]
    w_q = nc.dram_tensor(
        "WQP", [NP, 128, N_MO, 128], BF16, kind="ExternalInput"
    ).ap()
    w_k = nc.dram_tensor(
        "WKP", [NP, 128, N_MO, 128], BF16, kind="ExternalInput"
    ).ap()
    w_v = nc.dram_tensor("WV3", [128, N_MO, H * D], BF16, kind="ExternalInput").ap()
    # [mi=128, pair-chunk=8, dm=1024]
    w_o = nc.dram_tensor("WO3", [128, NP, DM], BF16, kind="ExternalInput").ap()
    out = nc.dram_tensor("out", [QH, DM], F32, kind="ExternalOutput").ap()
    if dbg:
        d_kt = nc.dram_tensor("d_kt", [128, S], BF16, kind="ExternalOutput").ap()
        d_qt = nc.dram_tensor("d_qt", [128, QH], BF16, kind="ExternalOutput").ap()
        d_vall = nc.dram_tensor(
            "d_vall", [128, N_SC * H * 65], BF16, kind="ExternalOutput"
        ).ap()
        d_onorm = nc.dram_tensor(
            "d_onorm", [128, QH], BF16, kind="ExternalOutput"
        ).ap()
        d_heads = nc.dram_tensor(
            "d_heads", [128, QH], BF16, kind="ExternalOutput"
        ).ap()

    with tile.TileContext(nc) as tc:
        with (
            tc.tile_pool(name="pers", bufs=1) as pers,
            tc.tile_pool(name="wkq", bufs=2) as wkq,
        ):
            ves = contextlib.ExitStack()
            vpool = ves.enter_context(tc.tile_pool(name="vpool", bufs=1))
            les = contextlib.ExitStack()
            ident_f32 = pers.tile([128, 128], F32)
            make_identity(nc, ident_f32[:])
            ident = pers.tile([128, 128], BF16)
            nc.vector.tensor_copy(ident[:], ident_f32[:])

            # v resident: per s-chunk block of 16 head-slots [1|v] (65 wide)
            v_all = pers.tile([128, N_SC, H, 65], BF16, tag="v_all")
            nc.vector.memset(v_all[:, :, :, 0:1], 1.0)
            # kt[p]: pair-stacked [2*64, S]; qt[p]: [2*64, QH] -> later
            # reused for normalized flipped heads, then pair-stacked headsT.
            kt_sb = [
                pers.tile([128, S], BF16, tag=f"kt{p}", name=f"kt{p}")
                for p in range(NP)
            ]
            qt_sb = [
                pers.tile([128, QH], BF16, tag=f"qt{p}", name=f"qt{p}")
                for p in range(NP)
            ]
            # flipped normalized heads, aliasing kt (dead after pair's scores)
            o_norm = [
                kt_sb[p][:, 0:1024].rearrange("p (s w) -> p s w", s=16)
                for p in range(NP)
            ]
            # resident staged transposed inputs for projections
            ktx = pers.tile([128, N_MO, S], BF16, tag="ktx")
            qtx = pers.tile([128, N_MO, QH], BF16, tag="qtx")
            vtx = [
                vpool.tile([128, N_MO, 128], BF16, tag=f"vt{sc}", name=f"vt{sc}")
                for sc in range(N_SC)
            ]
            wv_sb = vpool.tile([128, N_MO, H * D], BF16, tag="wv")

            cur_wk, cur_wq = {}, {}

            def prefetch_wk(p):
                wkt = wkq.tile([128, N_MO, 128], BF16, tag="wk", name="wk")
                cur_wk[p] = wkt
                nc.sync.dma_start(out=wkt[:], in_=w_k[:, :, p * 128 : (p + 1) * 128])

            def prefetch_wq(p):
                wqt = wkq.tile([128, N_MO, 128], BF16, tag="wq", name="wq")
                cur_wq[p] = wqt
                nc.sync.dma_start(out=wqt[:], in_=w_q[:, :, p * 128 : (p + 1) * 128])

            def dma_ktx(g):
                for mo in range(N_MO):
                    nc.sync.dma_start(
                        out=ktx[:, mo, g * 512 : (g + 1) * 512],
                        in_=kt_d[mo, :, g * 512 : (g + 1) * 512],
                    )

            def dma_wv(q):
                nc.sync.dma_start(
                    out=wv_sb[:, :, q * 256 : (q + 1) * 256],
                    in_=w_v[:, :, q * 256 : (q + 1) * 256],
                )

            # startup-critical DMA order: pair-0 K/Q weights + first K
            # columns + all of QT, then V / remaining K interleaved
            prefetch_wk(0)
            prefetch_wq(0)
            dma_ktx(0)
            for mo in range(N_MO):
                nc.sync.dma_start(out=qtx[:, mo, :], in_=qt_d[mo])
            dma_wv(0)
            nc.sync.dma_start(out=vtx[0][:], in_=vt_d[0])
            dma_ktx(1)
            nc.sync.dma_start(out=vtx[1][:], in_=vt_d[1])
            dma_ktx(2)
            dma_ktx(3)
            for q in range(1, 4):
                dma_wv(q)
            for sc in range(2, N_SC):
                nc.sync.dma_start(out=vtx[sc][:], in_=vt_d[sc])

            with (
                tc.tile_pool(name="psum_sp", bufs=1, space="PSUM") as spsum,
                tc.tile_pool(name="psum_o", bufs=1, space="PSUM") as opsum,
                tc.tile_pool(name="psum_pj", bufs=1, space="PSUM") as pjsum,
                tc.tile_pool(name="epool", bufs=5) as epool,
                tc.tile_pool(name="npool", bufs=1) as npool,
            ):
                # ---------- projection "filler" groups ----------
                def k_group(p, g):
                    pj = pjsum.tile([128, 512], F32, tag="pj")
                    for mo in range(N_MO):
                        nc.tensor.matmul(
                            pj[:],
                            cur_wk[p][:, mo, :],
                            ktx[:, mo, g * 512 : (g + 1) * 512],
                            start=(mo == 0),
                            stop=(mo == N_MO - 1),
                        )
                    nc.vector.tensor_copy(
                        kt_sb[p][:, g * 512 : (g + 1) * 512], pj[:]
                    )

                def q_group(p, g):
                    pj = pjsum.tile([128, 512], F32, tag="pj")
                    for mo in range(N_MO):
                        nc.tensor.matmul(
                            pj[:],
                            cur_wq[p][:, mo, :],
                            qtx[:, mo, g * 512 : (g + 1) * 512],
                            start=(mo == 0),
                            stop=(mo == N_MO - 1),
                        )
                    nc.vector.tensor_copy(
                        qt_sb[p][:, g * 512 : (g + 1) * 512], pj[:]
                    )

                def v_group(sc, quarter):
                    pj = pjsum.tile([128, 512], F32, tag="pj")
                    for mo in range(N_MO):
                        nc.tensor.matmul(
                            pj[:, 0:256],
                            vtx[sc][:, mo, :],
                            wv_sb[:, mo, quarter * 256 : (quarter + 1) * 256],
                            start=(mo == 0),
                            stop=(mo == N_MO - 1),
                        )
                    nc.vector.tensor_copy(
                        v_all[:, sc, quarter * 4 : (quarter + 1) * 4, 1:65],
                        pj[:, 0:256].rearrange("p (h w) -> p h w", h=4),
                    )

                # K/Q for pair 0 before attention starts (columns 0-511 at
                # least; the rest of K0 is the first filler work).
                k_group(0, 0)
                for g in range(2):
                    q_group(0, g)

                if dbg:
                    nc.sync.dma_start(out=d_kt[:], in_=kt_sb[0][:])
                    nc.sync.dma_start(out=d_qt[:], in_=qt_sb[0][:])

                # filler queue with prerequisites encoded by position:
                #   pair 0 steps: rest of K0, V quarter 0 (1/step), K1/Q1
                #   pair 1 steps: V quarter 1 (1/step), K2/Q2
                #   pairs 2-3:    V quarter 2 (1/2-step), K3/Q3, K4/Q4
                #   pairs 4-5:    V quarter 3 (1/2-step), K5-K7/Q5-Q7
                def kq_pair(p, prefetch=True):
                    out = (
                        [lambda p=p: prefetch_wk(p), lambda p=p: prefetch_wq(p)]
                        if prefetch
                        else []
                    )
                    for g in range(4):
                        out.append(lambda p=p, g=g: k_group(p, g))
                    for g in range(2):
                        out.append(lambda p=p, g=g: q_group(p, g))
                    return out

                def tp_half(p, half):
                    # PE re-transpose of normalized flipped heads (in the
                    # dead kt tile) into pair-stacked headsT in the qt tile
                    pjt = pjsum.tile([128, 512], F32, tag="pj", name="tp")
                    view = pjt[:].rearrange("p (a b) -> p a b", b=128)
                    for h in range(2):
                        for qi in range(4):
                            qc = half * 4 + qi
                            nc.tensor.matmul(
                                view[h * 64 : h * 64 + 64, qi, :],
                                o_norm[p][:, h * N_QC + qc, :],
                                ident[:],
                                start=True,
                                stop=True,
                                skip_group_check=True,
                            )
                    nc.vector.tensor_copy(
                        qt_sb[p][:, half * 512 : (half + 1) * 512], pjt[:]
                    )

                def tp_pair(p):
                    tp_half(p, 0)
                    tp_half(p, 1)

                def v_quarter(q):
                    return [
                        lambda sc=sc, q=q: v_group(sc, q) for sc in range(N_SC)
                    ]

                # per-pair filler schedules: list of lists (one per step)
                def spread(items, nsteps):
                    # distribute items across nsteps as evenly as possible
                    outl = [[] for _ in range(nsteps)]
                    for i, it in enumerate(items):
                        outl[(i * nsteps) // len(items)].append(it)
                    return outl

                sched = {}
                vq = [v_quarter(q) for q in range(4)]
                # pair 0: v quarter 0 must run at 1/step (o of (pair0, sc)
                # needs it by step sc+1); remaining K0 columns by step
                # 4*g-1; K1/Q1 anywhere inside pair 0.
                sched[0] = [[vq[0][0], vq[0][1], lambda: k_group(0, 1)]] + [
                    [vq[0][sc + 1]] for sc in range(1, N_SC - 1)
                ] + [[]]
                extras0 = [
                    lambda: k_group(0, 2),
                    lambda: k_group(0, 3),
                ] + kq_pair(1, prefetch=False)
                for i, f in enumerate(extras0):
                    sched[0][5 + i].append(f)
                # v quarter q's chunk sc is first consumed by pair 2q at
                # its step sc+1, so each quarter's second half can lag into
                # the consuming pair itself; this spreads filler evenly and
                # leaves only the transposes for pair 7's steps.
                sched[1] = spread(vq[1][:8] + kq_pair(2), N_SC)
                sched[2] = spread(vq[1][8:] + kq_pair(3), N_SC)
                sched[3] = spread(vq[2][:8] + kq_pair(4), N_SC)
                sched[4] = spread(vq[2][8:] + kq_pair(5), N_SC)
                sched[5] = spread(vq[3][:8] + kq_pair(6), N_SC)
                sched[6] = spread(vq[3][8:] + kq_pair(7), N_SC)
                sched[7] = spread(
                    [
                        lambda p=p, hf=hf: tp_half(p, hf)
                        for p in range(7)
                        for hf in (0, 1)
                    ],
                    N_SC,
                )

                # o accumulator slot -> AP. 16 slots (h,qc) packed into PSUM
                # banks as 7+7+2 (a [128,8,65] f32 tile would straddle a 2KB
                # bank boundary).
                def o_slot(tiles, h, qc):
                    s = h * N_QC + qc
                    if s < 7:
                        return tiles[0][:, s, :]
                    if s < 14:
                        return tiles[1][:, s - 7, :]
                    return tiles[2][:, s - 14, :]

                def emit_o(p, sc, h, o_ps, e_tiles):
                    e_sb = e_tiles.pop((sc, h))
                    for qc in range(N_QC):
                        s = h * N_QC + qc
                        # start_tensor_calc marks the whole 2KB PSUM bank
                        # pending-zero, so only the FIRST slot of each bank
                        # may set it; the other slots' first write then
                        # lands on pending-zero bytes (= overwrite).
                        nc.tensor.matmul(
                            o_slot(o_ps, h, qc),
                            e_sb[:, qc * 128 : (qc + 1) * 128],
                            v_all[:, sc, 2 * p + h, :],
                            start=(sc == 0 and s in (0, 7, 14)),
                            stop=(sc == N_SC - 1),
                            skip_group_check=True,
                        )

                # ---------------- attention (phase 2) ----------------
                for p in range(NP if 2 in phases else 0):
                    if p == 6:
                        # V work is done; swap the vt/wv space for W_O and
                        # prefetch it for the output projection
                        ves.close()
                        wop = les.enter_context(tc.tile_pool(name="wo", bufs=1))
                        wo_sb = wop.tile([128, NP, DM], BF16, tag="wo")
                        nc.sync.dma_start(
                            out=wo_sb[:, :, 0:512], in_=w_o[:, :, 0:512]
                        )
                        nc.sync.dma_start(
                            out=wo_sb[:, :, 512:1024], in_=w_o[:, :, 512:1024]
                        )
                    o_ps = [
                        opsum.tile([128, 7, 65], F32, tag="oA", name="oA"),
                        opsum.tile([128, 7, 65], F32, tag="oB", name="oB"),
                        opsum.tile([128, 2, 65], F32, tag="oC", name="oC"),
                    ]
                    e_tiles = {}
                    for sc in range(N_SC):
                        for h in range(2):
                            lo, hi = h * 64, h * 64 + 64
                            sp = spsum.tile([128, QH], F32, tag=f"sp{h}")
                            for qc2 in range(QH // 512):
                                nc.tensor.matmul(
                                    sp[:, qc2 * 512 : (qc2 + 1) * 512],
                                    kt_sb[p][lo:hi, sc * 128 : (sc + 1) * 128],
                                    qt_sb[p][lo:hi, qc2 * 512 : (qc2 + 1) * 512],
                                    start=True,
                                    stop=True,
                                    skip_group_check=True,
                                )
                            e_sb = epool.tile([128, QH], BF16, tag=f"e{h}")
                            nc.scalar.activation(
                                e_sb[:],
                                sp[:],
                                mybir.ActivationFunctionType.Exp,
                                scale=0.125,
                            )
                            e_tiles[(sc, h)] = e_sb
                        # value matmuls lag two s-chunks behind the scores so
                        # ACT has time to produce e without stalling PE
                        if sc > 1:
                            for h in range(2):
                                emit_o(p, sc - 2, h, o_ps, e_tiles)
                        for fill in sched[p][sc]:
                            fill()
                    for h in range(2):
                        emit_o(p, N_SC - 2, h, o_ps, e_tiles)
                    emit_o(p, N_SC - 1, 0, o_ps, e_tiles)
                    # copy PSUM -> SBUF (bf16) + f32 denominators; the oA
                    # bank only holds h=0 slots, so it can drain while the
                    # h=1 value matmuls still run
                    o_sb = npool.tile([128, 16, 65], BF16, tag="osb")
                    den = npool.tile([128, 16], F32, tag="den")
                    nc.vector.tensor_copy(o_sb[:, 0:7, :], o_ps[0][:])
                    nc.vector.tensor_copy(den[:, 0:7, None], o_ps[0][:, :, 0:1])
                    emit_o(p, N_SC - 1, 1, o_ps, e_tiles)
                    nc.vector.tensor_copy(o_sb[:, 7:14, :], o_ps[1][:])
                    nc.vector.tensor_copy(o_sb[:, 14:16, :], o_ps[2][:])
                    nc.vector.tensor_copy(den[:, 7:14, None], o_ps[1][:, :, 0:1])
                    nc.vector.tensor_copy(den[:, 14:16, None], o_ps[2][:, :, 0:1])
                    rec = npool.tile([128, 16], F32, tag="rec")
                    nc.vector.reciprocal_approx_fast(rec[:], den[:])
                    nc.vector.tensor_mul(
                        o_norm[p][:],
                        o_sb[:, :, 1:65],
                        rec[:, :, None].broadcast_to([128, 16, 64]),
                    )

                if 2 in phases:
                    tp_pair(7)
                if dbg:
                    nc.sync.dma_start(out=d_heads[:], in_=qt_sb[0][:])

                # drain unused fillers (for phases subsets)
                if 2 not in phases:
                    for p in range(NP):
                        for step in sched.get(p, []):
                            for fill in step:
                                fill()

            # ------------- head re-transpose + output proj -------------
            with (
                tc.tile_pool(name="psum_h", bufs=2, space="PSUM") as hpsum,
                tc.tile_pool(name="psum_f", bufs=3, space="PSUM") as fpsum,
                tc.tile_pool(name="fout", bufs=3) as fout,
            ):
                if 2 not in phases:
                    ves.close()
                    wop = les.enter_context(tc.tile_pool(name="wo", bufs=1))
                    wo_sb = wop.tile([128, NP, DM], BF16, tag="wo")
                    nc.sync.dma_start(out=wo_sb[:], in_=w_o[:])
                for p in range(NP if 2 in phases else 0):
                    h_ps = hpsum.tile([128, N_QC, 128], F32, tag="hps")
                    for h in range(2):
                        for qc in range(N_QC):
                            nc.tensor.matmul(
                                h_ps[h * 64 : h * 64 + 64, qc, :],
                                o_norm[p][:, h * N_QC + qc, :],
                                ident[:],
                                start=True,
                                stop=True,
                                skip_group_check=True,
                            )
                    nc.vector.tensor_copy(
                        qt_sb[p][:, 0:512],
                        h_ps[:, 0:4, :].rearrange("p a b -> p (a b)"),
                    )
                    nc.vector.tensor_copy(
                        qt_sb[p][:, 512:1024],
                        h_ps[:, 4:8, :].rearrange("p a b -> p (a b)"),
                    )

                for qc in range(N_QC if 3 in phases else 0):
                    for dmc in range(DM // 512):
                        fp = fpsum.tile([128, 512], F32, tag="fp")
                        for p in range(NP):
                            nc.tensor.matmul(
                                fp[:],
                                qt_sb[p][:, qc * 128 : (qc + 1) * 128],
                                wo_sb[:, p, dmc * 512 : (dmc + 1) * 512],
                                start=(p == 0),
                                stop=(p == NP - 1),
                            )
                        fo = fout.tile([128, 512], F32, tag="fo")
                        nc.scalar.copy(fo[:], fp[:])
                        nc.sync.dma_start(
                            out=out[
                                qc * 128 : (qc + 1) * 128,
                                dmc * 512 : (dmc + 1) * 512,
                            ],
                            in_=fo[:],
                        )
            les.close()
    nc.compile()
    return nc


_NC_CACHE = {}


def _get_nc():
    if "nc" not in _NC_CACHE:
        _NC_CACHE["nc"] = build()
    return _NC_CACHE["nc"]


def _prep_w3(w):
    # [H, DM, D] -> [mi=128, mo=8, (h d)=1024], bf16
    return np.ascontiguousarray(
        w.transpose(1, 0, 2).reshape(N_MO, 128, H * D).transpose(1, 0, 2)
    ).astype(ml_dtypes.bfloat16)


def _prep_w3p(w):
    # pair-major: [pair, mi=128, mo=8, 128]
    w3 = w.transpose(1, 0, 2).reshape(N_MO, 128, H * D).transpose(1, 0, 2)
    return np.ascontiguousarray(
        w3.reshape(128, N_MO, NP, 128).transpose(2, 0, 1, 3)
    ).astype(ml_dtypes.bfloat16)


def _prep_wo(w):
    # [H*D=1024, DM] -> [mi=128, chunk=8, DM], bf16
    return np.ascontiguousarray(
        w.reshape(NP, 128, DM).transpose(1, 0, 2)
    ).astype(ml_dtypes.bfloat16)


def _prep_xt(x):
    # [rows, DM] -> transposed [mo=8, 128, rows], bf16
    return np.ascontiguousarray(x.T.reshape(N_MO, 128, x.shape[0])).astype(
        ml_dtypes.bfloat16
    )


def _prep_vt(x):
    # [S, DM] -> [sc=16, 128(dm within mo), mo=8, 128(s within chunk)]
    # vt[sc, p, mo, c] = x[sc*128+c, mo*128+p]
    return np.ascontiguousarray(
        x.reshape(N_SC, 128, N_MO, 128).transpose(0, 3, 2, 1)
    ).astype(ml_dtypes.bfloat16)


def kernel(Q, K, V, W_Q, W_K, W_V, W_O, _trace=False):
    Q = np.asarray(Q, dtype=np.float32)
    K = np.asarray(K, dtype=np.float32)
    V = np.asarray(V, dtype=np.float32)
    wq = _prep_w3p(np.asarray(W_Q, dtype=np.float32))
    wk = _prep_w3p(np.asarray(W_K, dtype=np.float32))
    wv = _prep_w3(np.asarray(W_V, dtype=np.float32))
    wo = _prep_wo(np.asarray(W_O, dtype=np.float32))

    kt_b = [_prep_xt(K[b]) for b in range(B)]
    vt_b = [_prep_vt(V[b]) for b in range(B)]

    in_maps = []
    for c in range(N_CORES):
        b, half = c // 2, c % 2
        in_maps.append(
            {
                "QT": _prep_xt(Q[b, half * QH : (half + 1) * QH]),
                "KT": kt_b[b],
                "VTs": vt_b[b],
                "WQP": wq,
                "WKP": wk,
                "WV3": wv,
                "WO3": wo,
            }
        )

    nc = _get_nc()
    res = run_bass_kernel_spmd(nc, in_maps, list(range(N_CORES)), trace=_trace)
    out = np.empty((B, S, DM), dtype=np.float32)
    for c in range(N_CORES):
        b, half = c // 2, c % 2
        out[b, half * QH : (half + 1) * QH] = res.results[c]["out"]
    if _trace:
        kernel._last_results = res
    return out
